# revision 16
# baseline (speedup 1.0000x reference)
"""Memristor-crossbar linear layer on 8 Trainium2 NeuronCores.

Reference computation:
    inp   = dac(x * 0.15)                      # 8-bit DAC quantization
    planes= einsum('bi,pio->pbo', inp, w_pos - w_neg)
    q     = adc(planes)                        # ADC: scale 8020, round to 2^-8, clip +-16
    out   = einsum('pbo,p->bo', q, [4,2,1]) * 0.01 + bias

Approximations (error budget: harness gate is rel_err < 2e-2; measured 3.3e-3):
  1. Per-plane ADC rounding (step 2^-8, clip never active at 8.4 sigma) is
     dropped, collapsing the three bit-plane matmuls into ONE matmul with
     combined weights w_c = 4*w0 + 2*w1 + w2.  Output error std
     = 0.01 * step * sqrt(21/12) ~= 5e-5 -- negligible.
  2. Inputs (DAC integer levels k in [-127,127]) and combined weights are
     quantized to fp8e4m3 for the PE's DoubleRow mode, which sustains 2x
     the fp16 FLOP rate on this silicon (measured 216ns per
     [256k x 128b x 512o] matmul vs 216ns per half-size fp16 matmul).

Sharding: token-parallel (8192 -> 1024 tokens per core); weights replicated.
Natural [tokens, features] output layout on device; minimal aggregate HBM
traffic (w_c fp8 16.8MB + x slice 4MB + out fp16 8.4MB per core).

Per-core device kernel (1024 DoubleRow matmuls, ~221ns sustained each):
  - x tiles [128, 2, 1024] fp8 (16 = full k-range) and weight quarter tiles
    [128, 2, 1024] fp8 (64 = full w_c) in SBUF.  DoubleRow matmul:
    stationary x slice [128k, 2, 128b], moving weights [128k, 2, 512o],
    psum [128b, 512o]; 16 chained pair-steps cover the 4096 contraction.
    The per-matmul LDWEIGHTS (135ns) hides under the 216ns streams.
  - The PE executes its queue in order, so the first 8 accumulation chains
    are issued pair-step-major across all 8 psum banks, consuming each
    (w, x) tile pair exactly as its DMA lands (stream-matched prologue).
    All remaining chains are issued chain-serial so each chain's psum
    drain pipelines behind the next chain's matmuls.
  - Outputs go out on the gpsimd DMA queue -- the sync queue is busy with
    the 21MB input stream early on and would head-of-line block the
    epilogue (psum banks would back up into the PE).
  - Epilogue per chain: ScalarE Copy with fused scale (DAC volts, ADC
    scale, output factor, fp8 weight scale) psum->fp16, VectorE bias add.
  - 12 dummy warm-up matmuls bridge the DMA-boot window so the PE's HAM
    clock-gate reaches 2.4GHz with no re-throttling idle gap before the
    real work; the first x/w tiles are DMA'd in halves so the first real
    matmul's operands land as early as possible.
"""

import numpy as np
import ml_dtypes

TOKENS, D_IN, D_OUT = 8192, 4096, 4096
N_CORES = 8
B_PER = TOKENS // N_CORES         # 1024 tokens per core
P = 128                           # partition dim
KT2 = D_IN // (2 * P)             # 16 double-row pair steps
JB = B_PER // P                   # 8 token blocks per core
NQ = 4                            # weight quarter phases (streaming)
OGQ = 2                           # 512-wide output slices per quarter
WSCALE = 262144.0                 # 2^18: |w_c| < 7e-4 -> fp8 range (max ~183)
ALPHA = 0.6 * 8020.0 * 0.01 / (127.0 * WSCALE)

_BUILT = {}


def _build():
    if "nc" in _BUILT:
        return _BUILT["nc"]
    import concourse.mybir as mybir
    import concourse.tile as tile
    from concourse import bacc

    f32 = mybir.dt.float32
    f16 = mybir.dt.float16
    f8 = mybir.dt.float8e4
    DR = mybir.MatmulPerfMode.DoubleRow
    Copy = mybir.ActivationFunctionType.Copy

    nc = bacc.Bacc("TRN2", target_bir_lowering=False, debug=False,
                   num_devices=N_CORES)
    # Both inputs are host-rearranged so every stream DMA moves
    # >=1KB-contiguous runs on BOTH the DRAM and SBUF side: each
    # DMA_DIRECT2D trigger costs ~650ns of engine time regardless of size,
    # and descriptor-split triggers (from <1KB runs) serialize the stream.
    # x8: pair-major [kt2, kp, two, b];  w8: og-slice-major
    # [q, og, kt2, kp, two, oc].
    x8 = nc.dram_tensor("x8", [KT2, P, 2, B_PER], f8,
                        kind="ExternalInput").ap()
    w8 = nc.dram_tensor("w8", [NQ, OGQ, KT2, P, 2, 512], f8,
                        kind="ExternalInput").ap()
    biasb = nc.dram_tensor("biasb", [P, D_OUT], f16,
                           kind="ExternalInput").ap()
    out = nc.dram_tensor("out", [B_PER, D_OUT], f16,
                         kind="ExternalOutput").ap()
    QW = D_OUT // NQ              # 1024 output features per quarter

    with tile.TileContext(nc) as tc:
        with (
            tc.tile_pool(name="wpool", bufs=1) as wpool,
            tc.tile_pool(name="xpool", bufs=1) as xpool,
            tc.tile_pool(name="cpool", bufs=1) as cpool,
            tc.tile_pool(name="opool", bufs=12) as opool,
            tc.tile_pool(name="pspool", bufs=8, space="PSUM") as pspool,
        ):
            warm = cpool.tile([P, 512], f16, name="warm")
            nc.gpsimd.memset(warm[:], 0.0)
            warm_ps = pspool.tile([P, 512], f32, tag="ps", name="warm_ps")
            for _ in range(12):
                nc.tensor.matmul(warm_ps[:], warm[:, :P], warm[:],
                                 start=True, stop=True)

            # Input stream, in consumption order: the first og slice of the
            # weights interleaved with x per pair-step (the stream-matched
            # sub-round's whole working set, 6.3MB), then the remaining og
            # slices one at a time.  Each weight tile is DMA'd as two
            # og-slice halves; Tile's subtile dependency tracking lets a
            # matmul wait on just the half it reads.  Only the very first
            # tiles are split further so the DMA slow-start delivers the
            # first matmul's operands as early as possible.
            # x streams on the scalar engine's DMA queue, weights on the
            # sync queue: trigger issuance parallelizes across engines
            # (only sync/scalar/gpsimd can initiate DMAs).
            x_t = [None] * KT2
            w_t = [[None] * NQ for _ in range(KT2)]
            for kt2 in range(KT2):
                for q in range(NQ):
                    # og-major SBUF layout so each og-slice DMA writes
                    # contiguous per-partition runs
                    w_t[kt2][q] = wpool.tile([P, OGQ, 2, 512], f8,
                                             name=f"w_t_{kt2}_{q}")

            def load_w(kt2, q, og):
                nc.sync.dma_start(w_t[kt2][q][:, og], w8[q, og, kt2])

            load_w(0, 0, 0)
            x0 = xpool.tile([P, 2, B_PER], f8, name="x_t_0")
            nc.scalar.dma_start(x0[:, :, 0:512], x8[0][:, :, 0:512])
            nc.scalar.dma_start(x0[:, :, 512:B_PER],
                                x8[0][:, :, 512:B_PER])
            x_t[0] = x0
            for kt2 in range(1, KT2):
                load_w(kt2, 0, 0)
                xt = xpool.tile([P, 2, B_PER], f8, name=f"x_t_{kt2}")
                nc.scalar.dma_start(xt[:], x8[kt2])
                x_t[kt2] = xt
            bias_sb = cpool.tile([P, D_OUT], f16)
            nc.sync.dma_start(bias_sb[:], biasb[:])
            for q in range(NQ):
                for og in range(OGQ):
                    if q == 0 and og == 0:
                        continue
                    for kt2 in range(KT2):
                        load_w(kt2, q, og)

            def mm(ps, kt2, jb, q, og, start, stop):
                nc.tensor.matmul(
                    ps[:], x_t[kt2][:, :, jb * P:(jb + 1) * P],
                    w_t[kt2][q][:, og],
                    start=start, stop=stop, perf_mode=DR)

            def epilogue(ps, jb, og_abs, strips=1):
                oc = og_abs * 512
                o_sb = opool.tile([P, 512], f16, tag="o",
                                  name=f"o_{jb}_{og_abs}")
                W = 512 // strips
                for st in range(strips):
                    c = slice(st * W, (st + 1) * W)
                    nc.scalar.activation(o_sb[:, c], ps[:, c], Copy,
                                         bias=0.0, scale=ALPHA)
                    nc.vector.tensor_add(o_sb[:, c], o_sb[:, c],
                                         bias_sb[:, oc + st * W:
                                                 oc + (st + 1) * W])
                    nc.gpsimd.dma_start(
                        out[jb * P:(jb + 1) * P,
                            oc + st * W:oc + (st + 1) * W], o_sb[:, c])

            # Stream-matched first sub-round: 8 chains (all jb, quarter 0,
            # og 0) advance pair-step-major so the PE consumes each tile
            # pair as it lands instead of blocking on chain 0's tail.
            psA = [pspool.tile([P, 512], f32, tag="ps", name=f"psA_{jb}")
                   for jb in range(JB)]
            for kt2 in range(KT2 - 2):
                for jb in range(JB):
                    mm(psA[jb], kt2, jb, 0, 0,
                       start=(kt2 == 0), stop=False)
            # last two pair-steps chain-major: early chains stop (and their
            # psum banks drain) while the later chains' tails still run, so
            # the chain-serial phase never waits on a bank.
            for jb in range(JB):
                mm(psA[jb], KT2 - 2, jb, 0, 0, start=False, stop=False)
                mm(psA[jb], KT2 - 1, jb, 0, 0, start=False, stop=True)
                epilogue(psA[jb], jb, 0)

            # Remaining chains, serial: drains pipeline behind the next
            # chain's matmuls; data is resident (or streaming well ahead).
            for q in range(NQ):
                for og in range(OGQ):
                    if q == 0 and og == 0:
                        continue
                    for jb in range(JB):
                        last = (q == NQ - 1 and og == OGQ - 1
                                and jb == JB - 1)
                        ps = pspool.tile([P, 512], f32, tag="ps",
                                         name=f"ps_{q}_{og}_{jb}")
                        for kt2 in range(KT2):
                            mm(ps, kt2, jb, q, og,
                               start=(kt2 == 0), stop=(kt2 == KT2 - 1))
                        epilogue(ps, jb, q * OGQ + og,
                                 strips=4 if last else 1)
    nc.compile()
    _BUILT["nc"] = nc
    return nc


def _preprocess(x, w_pos, w_neg, bias):
    f32 = np.float32
    x = np.asarray(x, dtype=f32)
    w_pos = np.asarray(w_pos, dtype=f32)
    w_neg = np.asarray(w_neg, dtype=f32)
    bias = np.asarray(bias, dtype=f32)
    # DAC integer levels, transposed to [d_in, tokens], quantized to fp8,
    # pair-major [kt2, kp, two, tokens] (see _build)
    k = np.rint(np.clip(x * f32(0.15), f32(-1.0), f32(1.0)) * f32(127.0))
    x8 = np.ascontiguousarray(k.T).astype(ml_dtypes.float8_e4m3)
    x8 = x8.reshape(KT2, 2, P, TOKENS).transpose(0, 2, 1, 3)
    # combined bit-plane weights, scaled into fp8 range
    w_eff = w_pos - w_neg
    w_c = f32(4.0) * w_eff[0] + f32(2.0) * w_eff[1] + w_eff[2]
    w8 = (w_c * f32(WSCALE)).astype(ml_dtypes.float8_e4m3)
    # og-slice-major layout [q, og, kt2, kp, two, oc] (see _build)
    w8 = np.ascontiguousarray(
        w8.reshape(KT2, 2, P, NQ, OGQ, 512).transpose(3, 4, 0, 2, 1, 5))
    biasb = np.ascontiguousarray(
        np.broadcast_to(bias.astype(np.float16), (P, D_OUT)))
    in_maps = []
    for c in range(N_CORES):
        in_maps.append({
            "x8": np.ascontiguousarray(
                x8[:, :, :, c * B_PER:(c + 1) * B_PER]),
            "w8": w8,
            "biasb": biasb,
        })
    return in_maps


def run(inputs, trace=False, **kw):
    from concourse import bass_utils
    nc = _build()
    in_maps = _preprocess(inputs["x"], inputs["w_pos"], inputs["w_neg"],
                          inputs["bias"])
    res = bass_utils.run_bass_kernel_spmd(nc, in_maps,
                                          core_ids=list(range(N_CORES)),
                                          trace=trace, **kw)
    full = np.concatenate([res.results[c]["out"] for c in range(N_CORES)],
                          axis=0).astype(np.float32)
    return full, res


def kernel(**inputs):
    full, _ = run(inputs)
    return full
